# revision 16
# baseline (speedup 1.0000x reference)
"""Trainium2 Bass kernel for nn_BertCounterFactTransformer.

Contract: kernel(**inputs) takes FULL unsharded numpy inputs (as produced by
reference.setup_inputs()) and returns the FULL [32, 1024] float32 output.

Strategy (data-parallel over batch, 8 cores x 4 samples):
  - Host: compute false/option masks + per-sample-slot tile bounds from x_ids,
    transpose x to xT, shard over cores. The program is specialized to the
    bounds (max over cores per slot -> one SPMD program) and cached per
    bounds tuple; masks keep any extra computed tiles harmless.
  - Device, per sample (F = #false row tiles, OJ = first option col):
      qT projections      only cols [0, 128F)        (bf16 matmuls)
      kT projections      only cols [OJ, 512)
      gate                g = exp(al)*fmask / max(sum, 1e-8)
      scores              [128F, 512-OJ] block only   (3 types)
      E_sup = exp(S_sup/32 + obias), E_rep = exp(S_rep/32 + tanh(S_con/32) + obias)
      coeff_t = gate / rowsum(E_t);  r_t = coeff_t^T @ E_t
      pooled  = x^T @ [gate, r_rep, r_sup]  -> fused^T columns   (f32 matmuls)
  - Device, batched tail in f32: h=relu(W1^T fused + b1), y=W2^T h + b2, LN.

Key identity: gate @ (attn @ x) == (gate @ attn) @ x, so [L,D] attention
outputs are never materialized. Column masking is injected into PSUM via K=1
matmul bias rows (obias = -960 raw -> -30 after the 1/32 scale).
"""

import sys

if "/opt/trn_rl_repo" not in sys.path:
    sys.path.insert(0, "/opt/trn_rl_repo")

import numpy as np
import ml_dtypes
from contextlib import ExitStack

np_bf16 = ml_dtypes.bfloat16

import concourse.bacc as bacc
import concourse.bass as bass
import concourse.mybir as mybir
import concourse.tile as tile
from concourse import bass_utils

f32 = mybir.dt.float32
bf16 = mybir.dt.bfloat16
AF = mybir.ActivationFunctionType
ALU = mybir.AluOpType

B, L, D = 32, 512, 1024
NCORES = 8
BC = B // NCORES          # samples per core
NL = L // 128             # 4 L-tiles
ND = D // 128             # 8 D-tiles
NC3 = 3 * D // 128        # 24 tiles of the 3D fused dim
SCALE = 1.0 / 32.0        # 1/sqrt(D)
OBIAS_RAW = -960.0        # -30 after * SCALE
LN_EPS = 1e-5

PROJ_NAMES = ["w_sq", "w_sk", "w_cq", "w_ck", "w_rq", "w_rk"]
PBIAS_NAMES = ["b_sq", "b_sk", "b_cq", "b_ck", "b_rq", "b_rk"]
QS, KS, QC, KC, QR, KR = range(6)
QPROJ = (QS, QC, QR)

_PROGRAM_CACHE = {}
_M_CACHE = {}


def _m_matrix(wq, wk):
    import hashlib
    wq = np.asarray(wq, dtype=np.float32)
    wk = np.asarray(wk, dtype=np.float32)
    key = hashlib.blake2b(wq.tobytes() + wk.tobytes(), digest_size=16).digest()
    if key not in _M_CACHE:
        _M_CACHE[key] = np.ascontiguousarray(wq @ wk.T).astype(np_bf16)
    return _M_CACHE[key]


def build_program(bounds=((2, 2),) * BC, use_m=True, enable_asserts=False):
    """bounds[s] = (F, J0): false rows live in tiles [0,F), option cols in
    [128*J0, 512). Computing a superset is always correct (masks zero it)."""
    nc = bacc.Bacc(
        "TRN2",
        target_bir_lowering=False,
        debug=False,
        enable_asserts=enable_asserts,
        num_devices=NCORES,
    )

    xT_d = nc.dram_tensor("xT", [BC, D, L], bf16, kind="ExternalInput").ap()
    x_d = nc.dram_tensor("x", [BC, L, D], f32, kind="ExternalInput").ap()
    fmask_d = nc.dram_tensor("fmask", [BC, L], f32, kind="ExternalInput").ap()
    obias_d = nc.dram_tensor("obias", [BC, L], bf16, kind="ExternalInput").ap()

    if use_m:
        W_d = {p: nc.dram_tensor(n, [D, D], bf16, kind="ExternalInput").ap()
               for p, n in ((QS, "m_sup"), (QC, "m_con"), (QR, "m_rep"))}
    else:
        W_d = {p: nc.dram_tensor(PROJ_NAMES[p], [D, D], bf16, kind="ExternalInput").ap()
               for p in range(6)}
    Brow_d = {} if use_m else {
        p: nc.dram_tensor(PBIAS_NAMES[p], [1, D], bf16, kind="ExternalInput").ap()
        for p in range(6)}
    wanom_d = nc.dram_tensor("w_anom", [D, 1], bf16, kind="ExternalInput").ap()
    wf1_d = nc.dram_tensor("w_f1", [3 * D, D], f32, kind="ExternalInput").ap()
    wf2_d = nc.dram_tensor("w_f2", [D, D], f32, kind="ExternalInput").ap()
    bf1_d = nc.dram_tensor("b_f1", [128, ND], f32, kind="ExternalInput").ap()
    bf2_d = nc.dram_tensor("b_f2", [128, ND], f32, kind="ExternalInput").ap()
    lng_d = nc.dram_tensor("ln_g", [128, ND], f32, kind="ExternalInput").ap()
    lnb_d = nc.dram_tensor("ln_b", [128, ND], f32, kind="ExternalInput").ap()

    out_d = nc.dram_tensor("out", [BC, D], f32, kind="ExternalOutput").ap()

    with tile.TileContext(nc) as tc, ExitStack() as ctx:
        const_p = ctx.enter_context(tc.tile_pool(name="const", bufs=1))
        xT_p = ctx.enter_context(tc.tile_pool(name="xT", bufs=1))
        x_p = ctx.enter_context(tc.tile_pool(name="x", bufs=1))
        w_p = ctx.enter_context(tc.tile_pool(name="w", bufs=4))
        wf1_p = ctx.enter_context(tc.tile_pool(name="wf1", bufs=1))
        proj_p = ctx.enter_context(tc.tile_pool(name="proj", bufs=1))
        e_p = ctx.enter_context(tc.tile_pool(name="emat", bufs=1))
        tmp_p = ctx.enter_context(tc.tile_pool(name="tmp", bufs=2))
        sm_p = ctx.enter_context(tc.tile_pool(name="small", bufs=3))
        tail_p = ctx.enter_context(tc.tile_pool(name="tail", bufs=1))
        ps_big = ctx.enter_context(tc.tile_pool(name="psb", bufs=4, space="PSUM"))
        ps_s = ctx.enter_context(tc.tile_pool(name="pss", bufs=4, space="PSUM"))

        # ---- constants ----
        ones_row = const_p.tile([1, L], bf16)
        nc.vector.memset(ones_row[:], 1.0)
        ones_f = const_p.tile([1, 128], f32)
        nc.vector.memset(ones_f[:], 1.0)
        ones_col = const_p.tile([128, 1], f32)
        nc.vector.memset(ones_col[:], 1.0)

        wanom_t = const_p.tile([128, ND], bf16)
        nc.sync.dma_start(wanom_t[:], wanom_d[:, 0].rearrange("(k p) -> p k", p=128))
        brow_t = {}
        for p in Brow_d:
            brow_t[p] = const_p.tile([1, D], bf16, name=f"brow{p}")
            nc.sync.dma_start(brow_t[p][:], Brow_d[p][:])
        bf1_t = const_p.tile([128, ND], f32)
        nc.sync.dma_start(bf1_t[:], bf1_d[:])
        bf2_t = const_p.tile([128, ND], f32)
        nc.sync.dma_start(bf2_t[:], bf2_d[:])
        lng_t = const_p.tile([128, ND], f32)
        nc.sync.dma_start(lng_t[:], lng_d[:])
        lnb_t = const_p.tile([128, ND], f32)
        nc.sync.dma_start(lnb_t[:], lnb_d[:])

        fusedT = tail_p.tile([128, NC3, BC], f32)

        # per-slot geometry
        geo = []
        for s in range(BC):
            F, J0 = bounds[s]
            geo.append((F, J0, F * 128, J0 * 128, L - J0 * 128,
                        F > 0 and L - J0 * 128 > 0))

        # ---- all xT resident; projections with weights streamed ONCE ----
        xT_t = xT_p.tile([128, BC * ND, L], bf16)
        for s in range(BC):
            nc.sync.dma_start(
                xT_t[:, s * ND : (s + 1) * ND, :],
                xT_d[s].rearrange("(k p) i -> p k i", p=128),
            )

        # projs[p][s] tile slices: [128, ND, width_s]
        projs = [[None] * BC for _ in range(6)]
        proj_list = list(QPROJ) if use_m else list(range(6))
        for p in proj_list:
            qside = p in QPROJ
            widths = [
                ((g[2] if qside else g[4]) if g[5] else 0) for g in geo
            ]
            wmax = max(widths)
            if wmax == 0:
                continue
            pt = proj_p.tile([128, BC, ND, wmax], bf16, tag=f"proj{p}")
            for m in range(ND):
                wt = w_p.tile([128, ND, 128], bf16, tag="w")
                nc.sync.dma_start(
                    wt[:],
                    W_d[p][:, m * 128 : (m + 1) * 128].rearrange(
                        "(k p) c -> p k c", p=128
                    ),
                )
                for s in range(BC):
                    width = widths[s]
                    if width == 0:
                        continue
                    lo = 0 if qside else geo[s][3]
                    ps = ps_big.tile([128, width], f32, tag="ps")
                    for k in range(ND):
                        nc.tensor.matmul(
                            ps[:], lhsT=wt[:, k, :],
                            rhs=xT_t[:, s * ND + k, lo : lo + width],
                            start=(k == 0), stop=(use_m and k == ND - 1),
                        )
                    if not use_m:
                        nc.tensor.matmul(
                            ps[:], lhsT=brow_t[p][:, m * 128 : (m + 1) * 128],
                            rhs=ones_row[:, 0:width], start=False, stop=True,
                        )
                    nc.any.tensor_copy(pt[:, s, m, :], ps[:])
            for s in range(BC):
                if widths[s]:
                    projs[p][s] = pt

        for s in range(BC):
            F, J0, CQ, OJ, NO, have_attn = geo[s]

            x_t = x_p.tile([128, NL, D], f32)
            nc.sync.dma_start(x_t[:], x_d[s].rearrange("(t p) d -> p t d", p=128))
            fm_t = sm_p.tile([128, NL], f32, tag="fm")
            nc.sync.dma_start(fm_t[:], fmask_d[s].rearrange("(t p) -> p t", p=128))
            ob_t = sm_p.tile([1, L], bf16, tag="ob")
            nc.sync.dma_start(ob_t[:], obias_d[s : s + 1, :])

            # ---- anomaly logits -> gate (rows in tiles [0,F)) ----
            gate_t = sm_p.tile([128, NL], f32, tag="gate")
            if F > 0:
                ghat_t = sm_p.tile([128, NL], f32, tag="ghat")
                for it in range(F):
                    al_ps = ps_s.tile([128, 1], f32, tag="pss")
                    for k in range(ND):
                        nc.tensor.matmul(
                            al_ps[:],
                            lhsT=xT_t[:, s * ND + k, it * 128 : (it + 1) * 128],
                            rhs=wanom_t[:, k : k + 1],
                            start=(k == 0), stop=(k == ND - 1),
                        )
                    eg_t = sm_p.tile([128, 1], f32, tag="eg")
                    nc.scalar.activation(eg_t[:], al_ps[:], AF.Exp)
                    nc.vector.tensor_mul(
                        ghat_t[:, it : it + 1], eg_t[:], fm_t[:, it : it + 1]
                    )
                gsum_t = sm_p.tile([128, 1], f32, tag="gsum")
                nc.vector.tensor_reduce(
                    gsum_t[:], ghat_t[:, 0:F], axis=mybir.AxisListType.X, op=ALU.add
                )
                S_ps = ps_s.tile([1, 1], f32, tag="pss")
                nc.tensor.matmul(S_ps[:], lhsT=gsum_t[:], rhs=ones_col[:],
                                 start=True, stop=True)
                Smax_t = sm_p.tile([1, 1], f32, tag="Smax")
                nc.vector.tensor_scalar_max(Smax_t[:], S_ps[:], 1e-8)
                Sb_ps = ps_s.tile([128, 1], f32, tag="pss")
                nc.tensor.matmul(Sb_ps[:], lhsT=ones_f[:], rhs=Smax_t[:],
                                 start=True, stop=True)
                recipS_t = sm_p.tile([128, 1], f32, tag="recipS")
                nc.vector.reciprocal(recipS_t[:], Sb_ps[:])
                nc.vector.tensor_scalar_mul(gate_t[:, 0:F], ghat_t[:, 0:F],
                                            recipS_t[:])

            # ---- scores block [128F, NO] -> E, coeffs ----
            if have_attn:
                E_sup = e_p.tile([128, max(F, 1), NO], f32, tag="esup")
                E_rep = e_p.tile([128, max(F, 1), NO], f32, tag="erep")
                co_sup = sm_p.tile([128, NL], f32, tag="cosup")
                co_rep = sm_p.tile([128, NL], f32, tag="corep")
                for it in range(F):
                    isl = slice(it * 128, (it + 1) * 128)
                    ps_sup = ps_big.tile([128, NO], f32, tag="ps")
                    for k in range(ND):
                        nc.tensor.matmul(
                            ps_sup[:], lhsT=projs[QS][s][:, s, k, isl],
                            rhs=(xT_t[:, s * ND + k, OJ:L] if use_m else projs[KS][s][:, s, k, 0:NO]), start=(k == 0), stop=False,
                        )
                    nc.tensor.matmul(ps_sup[:], lhsT=ones_row[:, 0:128],
                                     rhs=ob_t[:, OJ:L], start=False, stop=True)
                    ps_con = ps_big.tile([128, NO], f32, tag="ps")
                    for k in range(ND):
                        nc.tensor.matmul(
                            ps_con[:], lhsT=projs[QC][s][:, s, k, isl],
                            rhs=(xT_t[:, s * ND + k, OJ:L] if use_m else projs[KC][s][:, s, k, 0:NO]),
                            start=(k == 0), stop=(k == ND - 1),
                        )
                    ps_rep = ps_big.tile([128, NO], f32, tag="ps")
                    for k in range(ND):
                        nc.tensor.matmul(
                            ps_rep[:], lhsT=projs[QR][s][:, s, k, isl],
                            rhs=(xT_t[:, s * ND + k, OJ:L] if use_m else projs[KR][s][:, s, k, 0:NO]), start=(k == 0), stop=False,
                        )
                    nc.tensor.matmul(ps_rep[:], lhsT=ones_row[:, 0:128],
                                     rhs=ob_t[:, OJ:L], start=False, stop=True)

                    T_t = tmp_p.tile([128, NO], f32, tag="T")
                    nc.scalar.activation(T_t[:], ps_con[:], AF.Tanh, scale=SCALE)
                    A_t = tmp_p.tile([128, NO], f32, tag="A")
                    nc.vector.scalar_tensor_tensor(
                        A_t[:], in0=ps_rep[:], scalar=SCALE, in1=T_t[:],
                        op0=ALU.mult, op1=ALU.add,
                    )
                    rs_sup = sm_p.tile([128, 1], f32, tag="rssup")
                    nc.scalar.activation(E_sup[:, it, :], ps_sup[:], AF.Exp,
                                         scale=SCALE, accum_out=rs_sup[:])
                    rs_rep = sm_p.tile([128, 1], f32, tag="rsrep")
                    nc.scalar.activation(E_rep[:, it, :], A_t[:], AF.Exp,
                                         accum_out=rs_rep[:])
                    rc_sup = sm_p.tile([128, 1], f32, tag="rcsup")
                    nc.vector.reciprocal(rc_sup[:], rs_sup[:])
                    nc.vector.tensor_mul(co_sup[:, it : it + 1],
                                         gate_t[:, it : it + 1], rc_sup[:])
                    rc_rep = sm_p.tile([128, 1], f32, tag="rcrep")
                    nc.vector.reciprocal(rc_rep[:], rs_rep[:])
                    nc.vector.tensor_mul(co_rep[:, it : it + 1],
                                         gate_t[:, it : it + 1], rc_rep[:])

            # ---- G = [gate, r_rep, r_sup] per row-tile ----
            G_t = sm_p.tile([128, NL, 3], f32, tag="G")
            nc.vector.memset(G_t[:], 0.0)
            if F > 0:
                for it in range(F):
                    nc.vector.tensor_copy(G_t[:, it, 0:1], gate_t[:, it : it + 1])
            if have_attn:
                for jt in range(J0, NL):
                    jsl = slice(jt * 128 - OJ, jt * 128 - OJ + 128)
                    r_ps = ps_s.tile([128, 2], f32, tag="pss")
                    for it in range(F):
                        nc.tensor.matmul(
                            r_ps[:, 0:1], lhsT=E_rep[:, it, jsl],
                            rhs=co_rep[:, it : it + 1],
                            start=(it == 0), stop=(it == F - 1),
                        )
                    for it in range(F):
                        nc.tensor.matmul(
                            r_ps[:, 1:2], lhsT=E_sup[:, it, jsl],
                            rhs=co_sup[:, it : it + 1],
                            start=(it == 0), stop=(it == F - 1),
                        )
                    nc.vector.tensor_copy(G_t[:, jt, 1:3], r_ps[:, 0:2])

            # ---- pooled vectors: x^T @ G -> fused^T columns (f32) ----
            rts = sorted(set(range(F)) | (set(range(J0, NL)) if have_attn else set()))
            if not rts:
                rts = [0]
            for m in range(ND):
                pool_ps = ps_s.tile([128, 3], f32, tag="pss")
                for i, rt in enumerate(rts):
                    nc.tensor.matmul(
                        pool_ps[:], lhsT=x_t[:, rt, m * 128 : (m + 1) * 128],
                        rhs=G_t[:, rt, :],
                        start=(i == 0), stop=(i == len(rts) - 1),
                    )
                for t in range(3):
                    nc.vector.tensor_copy(
                        fusedT[:, t * ND + m, s : s + 1], pool_ps[:, t : t + 1]
                    )

        # ---- batched MLP tail (f32) ----
        hT_t = tail_p.tile([128, ND, BC], f32)
        for m in range(ND):
            wt = wf1_p.tile([128, NC3, 128], f32, tag="wf1")
            nc.sync.dma_start(
                wt[:],
                wf1_d[:, m * 128 : (m + 1) * 128].rearrange("(k p) c -> p k c", p=128),
            )
            h_ps = ps_s.tile([128, BC], f32, tag="pss")
            for k in range(NC3):
                nc.tensor.matmul(h_ps[:], lhsT=wt[:, k, :], rhs=fusedT[:, k, :],
                                 start=(k == 0), stop=(k == NC3 - 1))
            nc.scalar.activation(hT_t[:, m, :], h_ps[:], AF.Relu,
                                 bias=bf1_t[:, m : m + 1])

        yT_t = tail_p.tile([128, ND, BC], f32)
        sq_t = tail_p.tile([128, ND, BC], f32)
        for m in range(ND):
            wt = wf1_p.tile([128, ND, 128], f32, tag="wf2")
            nc.sync.dma_start(
                wt[:],
                wf2_d[:, m * 128 : (m + 1) * 128].rearrange("(k p) c -> p k c", p=128),
            )
            y_ps = ps_s.tile([128, BC], f32, tag="pss")
            for k in range(ND):
                nc.tensor.matmul(y_ps[:], lhsT=wt[:, k, :], rhs=hT_t[:, k, :],
                                 start=(k == 0), stop=(k == ND - 1))
            nc.vector.tensor_scalar_add(yT_t[:, m, :], y_ps[:], bf2_t[:, m : m + 1])
            nc.scalar.square(sq_t[:, m, :], yT_t[:, m, :])

        sum_ps = ps_s.tile([1, BC], f32, tag="pss")
        for m in range(ND):
            nc.tensor.matmul(sum_ps[:], lhsT=ones_col[:], rhs=yT_t[:, m, :],
                             start=(m == 0), stop=(m == ND - 1))
        ssq_ps = ps_s.tile([1, BC], f32, tag="pss")
        for m in range(ND):
            nc.tensor.matmul(ssq_ps[:], lhsT=ones_col[:], rhs=sq_t[:, m, :],
                             start=(m == 0), stop=(m == ND - 1))
        mean_t = sm_p.tile([1, BC], f32, tag="mean")
        nc.scalar.mul(mean_t[:], sum_ps[:], 1.0 / D)
        msq_t = sm_p.tile([1, BC], f32, tag="msq")
        nc.scalar.mul(msq_t[:], ssq_ps[:], 1.0 / D)
        m2_t = sm_p.tile([1, BC], f32, tag="m2")
        nc.vector.tensor_mul(m2_t[:], mean_t[:], mean_t[:])
        var_t = sm_p.tile([1, BC], f32, tag="var")
        nc.vector.tensor_sub(var_t[:], msq_t[:], m2_t[:])
        nc.vector.tensor_scalar_add(var_t[:], var_t[:], LN_EPS)
        sd_t = sm_p.tile([1, BC], f32, tag="sd")
        nc.scalar.sqrt(sd_t[:], var_t[:])
        rstd_t = sm_p.tile([1, BC], f32, tag="rstd")
        nc.vector.reciprocal(rstd_t[:], sd_t[:])

        mb_ps = ps_s.tile([128, BC], f32, tag="pss")
        nc.tensor.matmul(mb_ps[:], lhsT=ones_f[:], rhs=mean_t[:],
                         start=True, stop=True)
        mb_t = sm_p.tile([128, BC], f32, tag="mbt")
        nc.vector.tensor_copy(mb_t[:], mb_ps[:])
        rb_ps = ps_s.tile([128, BC], f32, tag="pss")
        nc.tensor.matmul(rb_ps[:], lhsT=ones_f[:], rhs=rstd_t[:],
                         start=True, stop=True)
        rb_t = sm_p.tile([128, BC], f32, tag="rbt")
        nc.vector.tensor_copy(rb_t[:], rb_ps[:])

        for m in range(ND):
            z_t = tmp_p.tile([128, BC], f32, tag="z")
            nc.vector.tensor_sub(z_t[:], yT_t[:, m, :], mb_t[:])
            nc.vector.tensor_mul(z_t[:], z_t[:], rb_t[:])
            z2_t = tmp_p.tile([128, BC], f32, tag="z2")
            nc.vector.tensor_scalar(
                z2_t[:], z_t[:], scalar1=lng_t[:, m : m + 1],
                scalar2=lnb_t[:, m : m + 1], op0=ALU.mult, op1=ALU.add,
            )
            for s in range(BC):
                nc.sync.dma_start(out_d[s, m * 128 : (m + 1) * 128], z2_t[:, s : s + 1])

    nc.compile()
    return nc


def _host_prep(inputs):
    """Returns (in_maps, bounds)."""
    x = np.asarray(inputs["x"], dtype=np.float32)
    x_ids = np.asarray(inputs["x_ids"])
    pad_idx = int(np.asarray(inputs["pad_idx"]))
    sep_idx = int(np.asarray(inputs["sep_idx"]))
    assert x.shape == (B, L, D), x.shape

    valid = x_ids != pad_idx
    sepm = x_ids == sep_idx
    has = sepm.any(axis=1)
    first = sepm.argmax(axis=1)
    vlen = valid.sum(axis=1)
    fb = np.clip(vlen // 2, 1, max(1, L - 2))
    sp = np.where(has, first, fb)
    pos = np.arange(L)
    fmask = ((pos[None, :] < sp[:, None]) & valid).astype(np.float32)
    omask = (pos[None, :] > sp[:, None]) & valid
    obias = np.where(omask, 0.0, OBIAS_RAW).astype(np.float32)

    # per-slot tile bounds: F covers all false rows, J0 covers all option cols
    F_all = np.ceil(sp / 128).astype(int)           # false subset of [0, sep)
    J0_all = np.minimum((sp + 1) // 128, NL)        # option subset of [sep+1, L)
    bounds = tuple(
        (int(F_all.reshape(NCORES, BC)[:, s].max()),
         int(J0_all.reshape(NCORES, BC)[:, s].min()))
        for s in range(BC)
    )

    xT = np.ascontiguousarray(x.transpose(0, 2, 1))

    def w(name):
        return np.ascontiguousarray(np.asarray(inputs[name], dtype=np.float32))

    def ppart(name):
        return np.ascontiguousarray(np.asarray(inputs[name], dtype=np.float32)
                                    .reshape(ND, 128).T)

    use_m = all(not np.any(np.asarray(inputs[n])) for n in PBIAS_NAMES)
    shared = {}
    if use_m:
        for dst, qn, kn in (("m_sup", "w_sq", "w_sk"), ("m_con", "w_cq", "w_ck"),
                            ("m_rep", "w_rq", "w_rk")):
            shared[dst] = _m_matrix(inputs[qn], inputs[kn])
    else:
        for p in range(6):
            shared[PROJ_NAMES[p]] = w(PROJ_NAMES[p]).astype(np_bf16)
            shared[PBIAS_NAMES[p]] = w(PBIAS_NAMES[p]).reshape(1, D).astype(np_bf16)
    shared["w_anom"] = w("w_anom").reshape(D, 1).astype(np_bf16)
    shared["w_f1"] = w("w_f1")
    shared["w_f2"] = w("w_f2")
    shared["b_f1"] = ppart("b_f1")
    shared["b_f2"] = ppart("b_f2")
    shared["ln_g"] = ppart("ln_g")
    shared["ln_b"] = ppart("ln_b")

    in_maps = []
    for c in range(NCORES):
        sl = slice(c * BC, (c + 1) * BC)
        m = dict(shared)
        m["x"] = np.ascontiguousarray(x[sl])
        m["xT"] = np.ascontiguousarray(xT[sl]).astype(np_bf16)
        m["fmask"] = np.ascontiguousarray(fmask[sl])
        m["obias"] = np.ascontiguousarray(obias[sl]).astype(np_bf16)
        in_maps.append(m)
    return in_maps, bounds, use_m


def get_program(bounds, use_m):
    key = (bounds, use_m)
    if key not in _PROGRAM_CACHE:
        _PROGRAM_CACHE[key] = build_program(bounds, use_m=use_m)
    return _PROGRAM_CACHE[key]


def run(trace=False, **inputs):
    in_maps, bounds, use_m = _host_prep(inputs)
    nc = get_program(bounds, use_m)
    res = bass_utils.run_bass_kernel_spmd(
        nc, in_maps, core_ids=list(range(NCORES)), trace=trace
    )
    out = np.concatenate([res.results[c]["out"] for c in range(NCORES)], axis=0)
    return out.astype(np.float32), res


def kernel(**inputs):
    out, _ = run(trace=False, **inputs)
    return out
